# revision 1
# baseline (speedup 1.0000x reference)
"""Trainium2 Bass kernel for nn_LsqNonneg: batched NNLS.

Algorithm: constant-momentum accelerated projected gradient (converges to the
same NNLS KKT point the reference's 200-iteration FISTA approaches):

    AtA = A.T @ A;  L, mu = extreme eigenvalues;  step = 1/L
    W  = I - step*AtA;  beta = (sqrt(L/mu)-1)/(sqrt(L/mu)+1)
    B  = step * A.T @ X
    S_1 = relu(B); S_0 = 0
    for k = 1..K-1:
        S_{k+1} = relu( [(1+beta)W] S_k + [-beta W] S_{k-1} + B )
    return S_K

Both weight matrices are FIXED -> kept in SBUF, no per-iteration weight DMA.
fp32r matmuls round each operand to an 11-bit mantissa (measured); the
deterministic bias from rounding the fixed weights is suppressed by dithering:
n=8 pre-rounded variants per weight whose per-entry mean equals the exact
value, cycled in a balanced shuffled schedule. B is added into PSUM by the
vector engine in exact fp32 (cheaper than an ident@B matmul on the PE, which
runs fp32r at ~2 cycles/column on hardware).

Prologue: X is sent as bf16 (halves the dominant HBM transfer); step*A is an
exact bf16 hi/lo pair, so only X's rounding enters B and it averages out over
the 512-deep contraction.

Device layout (per core, ns=4096 columns): S packed [128, q=1024]; partition
group g holds columns [g*q,(g+1)*q). Weights are block-diagonal diag4 so one
full-array matmul advances all 4 groups. The loop is software-pipelined as 4
independent 256-column streams, each with its own PSUM ring and per-stream
state tiles: while one stream's PSUM waits on the VectorE B-add + ScalarE
relu, the PE runs the other streams' matmuls, keeping the PE array saturated
(and therefore un-throttled by PE_HAM).
"""

import os
import sys

import numpy as np

for _p in ("/opt/trn_rl_repo", "/root/.axon_site/_ro/trn_rl_repo"):
    if os.path.isdir(_p) and _p not in sys.path:
        sys.path.append(_p)

import ml_dtypes
from contextlib import ExitStack

import concourse.bass as bass
import concourse.bacc as bacc
import concourse.tile as tile
from concourse import mybir
from concourse.bass_utils import run_bass_kernel_spmd

M, KD, N_FULL, N_CORES = 512, 32, 32768, 8
ITERS = 60           # total iterations (S_ITERS is returned)
N_DITHER = 8
DITHER_SEED = 1
NSTR = 4             # independent column streams in the loop

F32 = mybir.dt.float32
F32R = mybir.dt.float32r
BF16 = mybir.dt.bfloat16

LAST_RESULTS = None  # BassKernelResults of the most recent run (for test.py)


def build_program(ns: int, iters: int, n_dither: int):
    q = ns // 4          # free extent of the packed [128, q] S layout
    qs = q // NSTR       # columns per stream
    nsl = q // 512       # 512-wide prologue slices
    assert ns % 2048 == 0 and nsl >= 1 and qs % 256 == 0

    nc = bacc.Bacc("TRN2", target_bir_lowering=False)

    x_d = nc.dram_tensor("x", [M, ns], BF16, kind="ExternalInput")
    apad_d = nc.dram_tensor("apad", [2, 4, M, 128], BF16, kind="ExternalInput")
    wd_d = nc.dram_tensor("wd", [n_dither, 2, 128, 128], F32,
                          kind="ExternalInput")
    id_d = nc.dram_tensor("ident", [128, 128], F32, kind="ExternalInput")
    out_d = nc.dram_tensor("s_out", [KD, ns], F32, kind="ExternalOutput")

    sched = _dither_schedule(iters, n_dither)

    with ExitStack() as ctx:
        tc = ctx.enter_context(tile.TileContext(nc))
        persist = ctx.enter_context(tc.tile_pool(name="persist", bufs=1))
        xpool = ctx.enter_context(tc.tile_pool(name="xstage", bufs=1))
        psum = ctx.enter_context(tc.tile_pool(name="psum", bufs=2,
                                              space="PSUM"))

        # ident goes first on the sync ring so the PE warm-up can start
        # within ~1us; the big weight DMAs ride the scalar ring (SWDGE/gpsimd
        # DMAs turned out to cost ~20us in end-of-program drains).
        id_sb = persist.tile([128, 128], F32R)
        nc.sync.dma_start(id_sb[:], id_d[:].bitcast(F32R))

        w_sb = persist.tile([128, 2 * n_dither * 128], F32R)
        nc.scalar.dma_start(
            w_sb[:].rearrange("p (i j m) -> p i j m", i=n_dither, j=2),
            wd_d[:].rearrange("i j p m -> p i j m").bitcast(F32R))

        # (hl, g, c) chunk of the bf16 hi/lo pair of step*A
        apc = persist.tile([128, 32 * 128], BF16)
        nc.scalar.dma_start(
            apc[:].rearrange("p (hl g c m) -> p hl g c m", hl=2, g=4, c=4),
            apad_d[:].rearrange("hl g (c p) m -> p hl g c m", p=128))

        # PE warm-up: PE_HAM keeps the array throttled (0.65-1.2 GHz) until
        # it has seen a few microseconds of sustained matmul activity.  Junk
        # matmuls on the already-arrived ident tile during the X-DMA window
        # release the throttle before the real prologue matmuls arrive.
        warm = psum.tile([128, 512], F32, name="warm", tag="pt3")
        for wi in range(12):
            nc.tensor.matmul(warm[:], id_sb[:],
                             w_sb[:, 0:512],
                             start=(wi == 0), stop=(wi == 11))

        b_sb = persist.tile([128, q], F32R)
        # per-(generation, stream) state tiles for exact dependency tracking
        s_st = [[persist.tile([128, qs], F32R, name=f"s{i}_{j}")
                 for j in range(NSTR)] for i in range(3)]
        sout = persist.tile([128, q], F32)

        # ---- prologue: B = (step A).T @ X in packed layout; S_1 = relu(B) ----
        # X row-chunk c, column-half h -> groups 2h, 2h+1
        xts = []
        for c in range(4):
            for h in range(2):
                xt = xpool.tile([128, ns // 2], BF16, name=f"xt{c}{h}")
                eng = nc.sync if (2 * c + h) % 2 == 0 else nc.scalar
                eng.dma_start(
                    xt[:],
                    x_d[128 * c:128 * (c + 1),
                        (ns // 2) * h:(ns // 2) * (h + 1)])
                xts.append(xt)
        pbs = [psum.tile([128, 512], F32, name=f"pb{s}", tag=f"pt{s % NSTR}")
               for s in range(nsl)]
        # every matmul writes the full 128-partition slice (zeros outside its
        # group block), so start/stop flags are per-slice across ALL writers.
        n_acc = [0] * nsl
        total_acc = 32
        for c in range(4):
            for h in range(2):
                xt = xts[2 * c + h]
                for g in (2 * h, 2 * h + 1):
                    for hl in range(2):
                        lhs = apc[:, 128 * (16 * hl + 4 * g + c):
                                  128 * (16 * hl + 4 * g + c + 1)]
                        for s in range(nsl):
                            i_acc = n_acc[s]
                            n_acc[s] = i_acc + 1
                            xoff = (g % 2) * q + 512 * s
                            nc.tensor.matmul(
                                pbs[s][:],
                                lhs,
                                xt[:, xoff:xoff + 512],
                                start=(i_acc == 0),
                                stop=(i_acc == total_acc - 1),
                            )
        spp = 512 // qs  # streams per prologue slice
        for s in range(nsl):
            nc.scalar.copy(b_sb[:, 512 * s:512 * (s + 1)], pbs[s][:])
            for jj in range(spp):
                j = spp * s + jj
                src = pbs[s][:, qs * jj:qs * (jj + 1)]
                if j % 2 == 0:
                    nc.vector.tensor_scalar_max(s_st[1][j][:], src, 0.0)
                else:
                    nc.scalar.activation(s_st[1][j][:], src,
                                         mybir.ActivationFunctionType.Relu)

        # ---- loop: k = 1..iters-1 computes S_{k+1} ----
        for k in range(1, iters):
            i = sched[k]
            wa = w_sb[:, 128 * (2 * i):128 * (2 * i + 1)]
            wb = w_sb[:, 128 * (2 * i + 1):128 * (2 * i + 2)]
            last = (k == iters - 1)
            for j in range(NSTR):
                cur = s_st[k % 3][j]
                prev = s_st[(k - 1) % 3][j]
                dest = (sout[:, qs * j:qs * (j + 1)] if last
                        else s_st[(k + 1) % 3][j][:])
                bsl = b_sb[:, qs * j:qs * (j + 1)]
                # full-bank psum tile (512 f32) so ring buffers never share a
                # bank (PE-write + engine-read same bank is fatal); only the
                # first qs columns are used.
                ptb = psum.tile([128, 512], F32, name=f"pt{k}_{j}",
                                tag=f"pt{j}")
                pt = ptb[:, 0:qs]
                # engine balance: stream 0 takes B via PE (ident@B) and its
                # relu on VectorE; streams 1-3 add B on VectorE and relu on
                # ScalarE.  This keeps PE/DVE/Act all near 2.0us/iter.
                if j == 0:
                    nc.tensor.matmul(pt, id_sb[:],
                                     b_sb[:, 0:qs],
                                     start=True, stop=False)
                    nc.tensor.matmul(pt, wa, cur[:],
                                     start=False, stop=(k == 1))
                    if k > 1:
                        nc.tensor.matmul(pt, wb, prev[:],
                                         start=False, stop=True)
                    nc.vector.tensor_scalar_max(dest, pt, 0.0)
                else:
                    nc.tensor.matmul(pt, wa, cur[:],
                                     start=True, stop=(k == 1))
                    if k > 1:
                        nc.tensor.matmul(pt, wb, prev[:],
                                         start=False, stop=True)
                    nc.vector.tensor_tensor(pt, pt, bsl.bitcast(F32),
                                            op=mybir.AluOpType.add)
                    nc.scalar.activation(dest, pt,
                                         mybir.ActivationFunctionType.Relu)

        for g in range(4):
            nc.sync.dma_start(out_d[:, g * q:(g + 1) * q],
                              sout[32 * g:32 * (g + 1), :])

    nc.finalize()
    return nc


def _dither_schedule(iters, n):
    sched = np.concatenate([np.arange(n)] * (iters // n + 2))[:iters]
    rng = np.random.default_rng(DITHER_SEED)
    rng.shuffle(sched)
    return sched


def _round11(x):
    u = np.ascontiguousarray(np.asarray(x, dtype=np.float32)).view(np.uint32)
    u = ((u + np.uint32(1 << 11)) >> np.uint32(12)) << np.uint32(12)
    return u.view(np.float32).astype(np.float64)


def _bf16(x):
    return np.asarray(x, dtype=np.float32).astype(ml_dtypes.bfloat16)


def _dither_variants(Mx, n):
    """n 11-bit-exact matrices whose per-entry mean ~= Mx."""
    M64 = np.asarray(Mx, dtype=np.float64)
    hi = _round11(M64)
    ulp = 2.0 ** (np.floor(np.log2(np.abs(M64) + 1e-300)) - 11)
    flo = np.where(hi > M64, hi - ulp, hi)
    fhi = flo + ulp
    frac = np.clip((M64 - flo) / ulp, 0, 1)
    cnt = np.rint(frac * n).astype(int)
    return [np.where(i < cnt, fhi, flo).astype(np.float32) for i in range(n)]


def host_prep(A: np.ndarray, n_dither: int):
    A64 = np.asarray(A, dtype=np.float64)
    AtA = A64.T @ A64
    ev = np.linalg.eigvalsh(AtA)
    L, mu = ev[-1], ev[0]
    step = 1.0 / L
    W = np.eye(KD) - step * AtA
    beta = (np.sqrt(L / mu) - 1.0) / (np.sqrt(L / mu) + 1.0)

    was = _dither_variants(((1.0 + beta) * W).T, n_dither)
    wbs = _dither_variants((-beta * W).T, n_dither)
    wd = np.zeros((n_dither, 2, 128, 128), dtype=np.float32)
    for i in range(n_dither):
        for g in range(4):
            blk = slice(32 * g, 32 * (g + 1))
            wd[i, 0][blk, blk] = was[i]
            wd[i, 1][blk, blk] = wbs[i]

    As = (step * A64).astype(np.float32).astype(np.float64)
    As_h = _bf16(As)
    As_l = _bf16(As - As_h.astype(np.float64))
    apad = np.zeros((2, 4, M, 128), dtype=ml_dtypes.bfloat16)
    for g in range(4):
        apad[0, g, :, 32 * g:32 * (g + 1)] = As_h
        apad[1, g, :, 32 * g:32 * (g + 1)] = As_l
    ident = np.eye(128, dtype=np.float32)
    return wd, apad, ident


_PROGRAM_CACHE = {}


def _get_program(ns, iters, n_dither):
    key = (ns, iters, n_dither)
    if key not in _PROGRAM_CACHE:
        _PROGRAM_CACHE[key] = build_program(ns, iters, n_dither)
    return _PROGRAM_CACHE[key]


def kernel(X: np.ndarray, A: np.ndarray) -> np.ndarray:
    global LAST_RESULTS
    X = np.ascontiguousarray(np.asarray(X, dtype=np.float32))
    A = np.ascontiguousarray(np.asarray(A, dtype=np.float32))
    assert X.shape == (M, N_FULL) and A.shape == (M, KD)

    ns = N_FULL // N_CORES
    wd, apad, ident = host_prep(A, N_DITHER)
    nc = _get_program(ns, ITERS, N_DITHER)

    Xb = _bf16(X)
    in_maps = []
    for c in range(N_CORES):
        in_maps.append({
            "x": np.ascontiguousarray(Xb[:, c * ns:(c + 1) * ns]),
            "apad": apad,
            "wd": wd,
            "ident": ident,
        })

    res = run_bass_kernel_spmd(nc, in_maps, core_ids=list(range(N_CORES)))
    LAST_RESULTS = res
    S = np.concatenate([res.results[c]["s_out"] for c in range(N_CORES)], axis=1)
    return np.ascontiguousarray(S.astype(np.float32))



# revision 6
# speedup vs baseline: 1.5315x; 1.5315x over previous
"""Trainium2 Bass kernel for nn_LsqNonneg: batched NNLS.

Algorithm: projected heavy-ball on the V-substitution of the NNLS KKT
iteration.  With Wa = (1+b)I - a*AtA, Wb = -b*I (heavy ball, a,b tuned from
the extreme eigenvalues of AtA), the S-iteration

    S_{k+1} = relu(Wa S_k - b S_{k-1} + a AtX)

becomes, under V := S - Sunc with Sunc = (AtA)^-1 AtX  (E := -Sunc):

    V_{k+1} = max(Wa V_k - b V_{k-1}, E),     S = V - E

i.e. the per-iteration bias add AND the relu collapse into a single
tensor_tensor(max) against the fixed threshold tile E.  Heavy ball's rate
(sqrt(k)-1)/(sqrt(k)+1) ~= 0.86/iter vs Nesterov's 1-1/sqrt(k) ~= 0.92
roughly halves the iteration count vs the FISTA-form baseline.

Precision: matmuls run bf16 (1 cyc/col vs fp32r's ~2).  Wa mixes O(1)
identity structure with the O(0.1) -a*AtA payload, so rounding Wa whole to
bf16 would inject ~ulp(1.7) noise each iteration; instead Wa is split as
c*I + Wa~ with c an exact-bf16 scalar near its mean diagonal: the c*ident
and -b*ident matmuls are exact in bf16 (products of exact bf16 values), and
Wa~'s entries are all <~0.11 so its bf16 rounding noise is ~20x smaller.
Wa~ is further dithered over 8 pre-rounded variants (per-entry mean equals
the exact value) cycled in a shuffled schedule.  States V are stored bf16;
E stays f32 (it biases the fixed point 1:1).  alpha is backed off 5% from
the heavy-ball optimum: the optimum sits exactly on the lambda=L stability
edge and weight rounding must not tip it over.

Prologue: E = (-A(AtA)^-1).T @ X with bf16 X and per-row-chunk-dithered
bf16 lhs, single pass (no hi/lo): 16 accumulating matmuls per 512-col
slice.  All weight/constant tensors are HOST-PACKED into their exact SBUF
layouts so every DMA is a contiguous per-partition copy (the baseline's
strided weight-DMA rearrange held the PE idle for ~15us).  X rides 4 DMA
rings (sync/scalar/vector/tensor) to hit full HBM bandwidth.

Device layout (per core, ns=4096 cols): packed [128, q=1024]; partition
group g holds original columns [g*q,(g+1)*q).  The loop runs 4 independent
256-col streams, each with its own PSUM ring (pool bufs=2 x 4 tags = all 8
banks) and 3-deep state ring; the fused max alternates DVE / Pool(gpsimd)
by stream, leaving Act free for prologue copies and the epilogue
S = relu(P_last - E).
"""

import os
import sys

import numpy as np

for _p in ("/opt/trn_rl_repo", "/root/.axon_site/_ro/trn_rl_repo"):
    if os.path.isdir(_p) and _p not in sys.path:
        sys.path.append(_p)

import ml_dtypes
from contextlib import ExitStack

import concourse.bass as bass
import concourse.bacc as bacc
import concourse.tile as tile
from concourse import mybir
from concourse.bass_utils import run_bass_kernel_spmd

M, KD, N_FULL, N_CORES = 512, 32, 32768, 8
ITERS = 40           # loop computes V_2..V_ITERS; V_ITERS is returned
N_DITHER = 8
DITHER_SEED = 1
NSTR = 4             # independent column streams in the loop
ALPHA_BACKOFF = 0.95

F32 = mybir.dt.float32
BF16 = mybir.dt.bfloat16

LAST_RESULTS = None  # BassKernelResults of the most recent run (for test.py)


def build_program(ns: int, iters: int, n_dither: int):
    q = ns // 4          # free extent of the packed [128, q] layout
    qs = q // NSTR       # columns per stream
    nsl = q // 512       # 512-wide prologue slices
    assert ns % 2048 == 0 and nsl >= 1 and qs % 256 == 0

    nc = bacc.Bacc("TRN2", target_bir_lowering=False)

    x_d = nc.dram_tensor("x", [M, ns], BF16, kind="ExternalInput")
    ppad_d = nc.dram_tensor("ppad", [128, 16 * 128], BF16,
                            kind="ExternalInput")
    wd_d = nc.dram_tensor("wd", [128, n_dither * 128], BF16,
                          kind="ExternalInput")
    cid_d = nc.dram_tensor("cid", [128, 128], BF16, kind="ExternalInput")
    bid_d = nc.dram_tensor("bid", [128, 128], BF16, kind="ExternalInput")
    out_d = nc.dram_tensor("s_out", [KD, ns], F32, kind="ExternalOutput")

    sched = _dither_schedule(iters, n_dither)

    with ExitStack() as ctx:
        tc = ctx.enter_context(tile.TileContext(nc))
        persist = ctx.enter_context(tc.tile_pool(name="persist", bufs=1))
        xpool = ctx.enter_context(tc.tile_pool(name="xstage", bufs=1))
        psum = ctx.enter_context(tc.tile_pool(name="psum", bufs=2,
                                              space="PSUM"))

        # cident first on the sync ring: the PE warm-up depends only on it.
        cid_sb = persist.tile([128, 128], BF16)
        nc.sync.dma_start(cid_sb[:], cid_d[:])
        bid_sb = persist.tile([128, 128], BF16)
        nc.sync.dma_start(bid_sb[:], bid_d[:])
        w_sb = persist.tile([128, n_dither * 128], BF16)
        nc.sync.dma_start(w_sb[:], wd_d[:])
        # prologue lhs (-A(AtA)^-1 chunks, group-padded) on the scalar ring
        pp_sb = persist.tile([128, 16 * 128], BF16)
        nc.scalar.dma_start(pp_sb[:], ppad_d[:])

        # PE warm-up: junk matmuls on cident release the PE_HAM throttle
        # during the X-DMA window.
        warm = psum.tile([128, 512], F32, name="warm", tag="pt1")
        for wi in range(24):
            nc.tensor.matmul(warm[:, 0:128], cid_sb[:], cid_sb[:],
                             start=(wi == 0), stop=(wi == 23))

        # X row-chunk c, column-half h (groups 2h, 2h+1); only SP/Act have
        # HW DMA queues, and each models ~330GB/s so 2 saturate HBM.
        rings = [nc.sync, nc.scalar]
        xts = {}
        for c in range(4):
            for h in range(2):
                xt = xpool.tile([128, ns // 2], BF16, name=f"xt{c}{h}")
                rings[(2 * c + h) % 2].dma_start(
                    xt[:],
                    x_d[128 * c:128 * (c + 1),
                        (ns // 2) * h:(ns // 2) * (h + 1)])
                xts[(c, h)] = xt

        e_sb = persist.tile([128, q], F32)
        v_st = [[persist.tile([128, qs], BF16, name=f"v{i}_{j}")
                 for j in range(NSTR)] for i in range(3)]
        sout = persist.tile([128, q], F32)

        # ---- prologue: E = (-A(AtA)^-1).T @ X in packed layout ----
        pbs = [psum.tile([128, 512], F32, name=f"pb{s}", tag=f"pt{2 * s}")
               for s in range(nsl)]
        n_acc = [0] * nsl
        for c in range(4):
            for h in range(2):
                xt = xts[(c, h)]
                for g in (2 * h, 2 * h + 1):
                    lhs = pp_sb[:, 128 * (4 * c + g):128 * (4 * c + g + 1)]
                    for s in range(nsl):
                        i_acc = n_acc[s]
                        n_acc[s] = i_acc + 1
                        xoff = (g % 2) * q + 512 * s
                        nc.tensor.matmul(
                            pbs[s][:], lhs, xt[:, xoff:xoff + 512],
                            start=(i_acc == 0), stop=(i_acc == 15))
        spp = 512 // qs  # streams per prologue slice
        for s in range(nsl):
            nc.scalar.copy(e_sb[:, 512 * s:512 * (s + 1)], pbs[s][:])
            for jj in range(spp):
                j = spp * s + jj
                # V1 = relu(E); gpsimd can't see PSUM, so it reads the
                # e_sb copy while DVE reads PSUM directly.
                if j % 2 == 0:
                    src = pbs[s][:, qs * jj:qs * (jj + 1)]
                    nc.vector.tensor_scalar_max(v_st[1][j][:], src, 0.0)
                else:
                    src = e_sb[:, qs * j:qs * (j + 1)]
                    nc.gpsimd.tensor_scalar_max(v_st[1][j][:], src, 0.0)

        # ---- loop: k = 1..iters-1 computes V_{k+1} ----
        for k in range(1, iters):
            wsl = w_sb[:, 128 * sched[k]:128 * (sched[k] + 1)]
            last = (k == iters - 1)
            for j in range(NSTR):
                cur = v_st[k % 3][j]
                prev = v_st[(k - 1) % 3][j] if k > 1 else v_st[1][j]
                ptb = psum.tile([128, 512], F32, name=f"pt{k}_{j}",
                                tag=f"pt{j}")
                pt = ptb[:, 0:qs]
                esl = e_sb[:, qs * j:qs * (j + 1)]
                # -b*V_{k-1} first: it never waits on the newest state
                nc.tensor.matmul(pt, bid_sb[:], prev[:],
                                 start=True, stop=False)
                nc.tensor.matmul(pt, wsl, cur[:], start=False, stop=False)
                nc.tensor.matmul(pt, cid_sb[:], cur[:],
                                 start=False, stop=True)
                # only DVE (and Act) may touch PSUM
                if last:
                    # S = max(P,E)-E = relu(P-E): sub in PSUM, relu on Act
                    nc.vector.tensor_tensor(pt, pt, esl,
                                            op=mybir.AluOpType.subtract)
                    nc.scalar.activation(sout[:, qs * j:qs * (j + 1)], pt,
                                         mybir.ActivationFunctionType.Relu)
                else:
                    nc.vector.tensor_tensor(v_st[(k + 1) % 3][j][:], pt, esl,
                                            op=mybir.AluOpType.max)

        for g in range(4):
            nc.sync.dma_start(out_d[:, g * q:(g + 1) * q],
                              sout[32 * g:32 * (g + 1), :])

    nc.finalize()
    return nc


def _dither_schedule(iters, n):
    sched = np.concatenate([np.arange(n)] * (iters // n + 2))[:iters]
    rng = np.random.default_rng(DITHER_SEED)
    rng.shuffle(sched)
    return sched


def _bf16(x):
    return np.asarray(x, dtype=np.float32).astype(ml_dtypes.bfloat16)


def _bf16_val(x):
    return float(np.float32(ml_dtypes.bfloat16(np.float32(x))))


def _dither_variants(Mx, n):
    """n bf16-exact matrices whose per-entry mean ~= Mx."""
    M64 = np.asarray(Mx, dtype=np.float64)
    hi = _bf16(M64).astype(np.float64)
    ulp = 2.0 ** (np.floor(np.log2(np.abs(M64) + 1e-300)) - 8)
    flo = np.where(hi > M64, hi - ulp, hi)
    fhi = flo + ulp
    frac = np.clip((M64 - flo) / ulp, 0, 1)
    cnt = np.rint(frac * n).astype(int)
    return [np.where(i < cnt, fhi, flo).astype(np.float32) for i in range(n)]


def host_prep(A: np.ndarray, n_dither: int):
    A64 = np.asarray(A, dtype=np.float64)
    AtA = A64.T @ A64
    ev = np.linalg.eigvalsh(AtA)
    L, mu = ev[-1], ev[0]
    kap = L / mu
    alpha = ALPHA_BACKOFF * 4.0 / (np.sqrt(L) + np.sqrt(mu)) ** 2
    beta = _bf16_val(((np.sqrt(kap) - 1.0) / (np.sqrt(kap) + 1.0)) ** 2)
    Wa = (1.0 + beta) * np.eye(KD) - alpha * AtA
    c = _bf16_val(1.0 + beta - alpha * np.mean(np.diag(AtA)))
    Wt = Wa - c * np.eye(KD)

    wvars = _dither_variants(Wt.T, n_dither)
    wd = np.zeros((128, n_dither * 128), dtype=ml_dtypes.bfloat16)
    for i in range(n_dither):
        for g in range(4):
            wd[32 * g:32 * (g + 1),
               128 * i + 32 * g:128 * i + 32 * (g + 1)] = _bf16(wvars[i])

    cid = np.zeros((128, 128), dtype=ml_dtypes.bfloat16)
    np.fill_diagonal(cid, np.float32(c))
    bid = np.zeros((128, 128), dtype=ml_dtypes.bfloat16)
    np.fill_diagonal(bid, np.float32(-beta))

    # prologue lhs: -A(AtA)^-1 row-chunks, per-chunk dithered bf16,
    # group-padded: ppad[:, 128*(4c+g)+32g : +32] = chunk c of -P
    P = A64 @ np.linalg.inv(AtA)
    ppad = np.zeros((128, 16 * 128), dtype=ml_dtypes.bfloat16)
    for cch in range(4):
        Pc = -P[128 * cch:128 * (cch + 1), :]
        pv = _dither_variants(Pc, 4)[cch % 4]
        for g in range(4):
            ppad[:, 128 * (4 * cch + g) + 32 * g:
                 128 * (4 * cch + g) + 32 * (g + 1)] = _bf16(pv)
    return wd, ppad, cid, bid


_PROGRAM_CACHE = {}


def _get_program(ns, iters, n_dither):
    key = (ns, iters, n_dither)
    if key not in _PROGRAM_CACHE:
        _PROGRAM_CACHE[key] = build_program(ns, iters, n_dither)
    return _PROGRAM_CACHE[key]


def kernel(X: np.ndarray, A: np.ndarray) -> np.ndarray:
    global LAST_RESULTS
    X = np.ascontiguousarray(np.asarray(X, dtype=np.float32))
    A = np.ascontiguousarray(np.asarray(A, dtype=np.float32))
    assert X.shape == (M, N_FULL) and A.shape == (M, KD)

    ns = N_FULL // N_CORES
    wd, ppad, cid, bid = host_prep(A, N_DITHER)
    nc = _get_program(ns, ITERS, N_DITHER)

    Xb = _bf16(X)
    in_maps = []
    for c in range(N_CORES):
        in_maps.append({
            "x": np.ascontiguousarray(Xb[:, c * ns:(c + 1) * ns]),
            "ppad": ppad,
            "wd": wd,
            "cid": cid,
            "bid": bid,
        })

    res = run_bass_kernel_spmd(nc, in_maps, core_ids=list(range(N_CORES)))
    LAST_RESULTS = res
    S = np.concatenate([res.results[c]["s_out"] for c in range(N_CORES)],
                       axis=1)
    return np.ascontiguousarray(S.astype(np.float32))


# revision 8
# speedup vs baseline: 1.6736x; 1.0928x over previous
"""Trainium2 Bass kernel for nn_LsqNonneg: batched NNLS.

Algorithm: projected Chebyshev/heavy-ball on the V-substitution of the NNLS
KKT iteration.  With Wa_k = (1+b_k)I - a_k*AtA, the S-iteration

    S_{k+1} = relu(Wa_k S_k - b_k S_{k-1} + a_k AtX)

becomes, under V := S - Sunc with Sunc = (AtA)^-1 AtX  (E := -Sunc):

    V_{k+1} = max(Wa_k V_k - b_k V_{k-1}, E),     S = V - E

i.e. the per-iteration bias add AND the relu collapse into a single DVE
tensor_tensor(max) against the fixed threshold tile E (computed in the
prologue as (-A(AtA)^-1).T @ X -- same cost as computing AtX).

Schedule: 12 Chebyshev ramp iterations (exact [mu, L] from the host
eigendecomposition of AtA) then constant heavy-ball at the optimum
(alpha backed off 5% from the lambda=L stability edge).  Chebyshev's
transient reaches the same error ~12 iterations earlier than constant
momentum from the warm start V_1 = relu(E).

Precision: phase 1 (24 updates) runs bf16 weights+states -- its ~1e-2
bf16-state-rounding floor is then crushed by phase 2: 8 fp32r polish
updates on f32 states (fp32r noise ~2^-12), contracting the bf16-phase
noise by rho^8 while the weights carry 11-bit-dithered variants.  E stays
f32 throughout (it biases the fixed point 1:1).  The phase transition is
seamless: each matmul picks the weight flavor matching its state operand's
dtype, so V_25 (bf16) and V_26 (f32) coexist inside one update.

All weight tensors are host-packed into exact SBUF layouts (contiguous
per-partition DMAs); X rides both HW DMA queues (SP+Act) right behind the
small ramp weights, and the late-phase weights trail the X chunks since
they are not needed until ~35us in.
"""

import os
import sys

import numpy as np

for _p in ("/opt/trn_rl_repo", "/root/.axon_site/_ro/trn_rl_repo"):
    if os.path.isdir(_p) and _p not in sys.path:
        sys.path.append(_p)

import ml_dtypes
from contextlib import ExitStack

import concourse.bass as bass
import concourse.bacc as bacc
import concourse.tile as tile
from concourse import mybir
from concourse.bass_utils import run_bass_kernel_spmd

M, KD, N_FULL, N_CORES = 512, 32, 32768, 8
N_RAMP = 12          # Chebyshev ramp updates (per-iteration weights)
N_BF16 = 24          # total bf16-phase updates (incl. ramp)
N_POLISH = 8         # fp32r polish updates
TOTAL_UPD = N_BF16 + N_POLISH
N_DITHER = 8         # bf16 constant-phase dither variants
N_DITHER32 = 4       # fp32r-phase dither variants
DITHER_SEED = 1
NSTR = 4             # independent column streams in the loop
ALPHA_BACKOFF = 0.95

F32 = mybir.dt.float32
F32R = mybir.dt.float32r
BF16 = mybir.dt.bfloat16

LAST_RESULTS = None  # BassKernelResults of the most recent run (for test.py)


def _sched(n, lo, cnt, seed):
    s = np.concatenate([np.arange(cnt)] * (n // cnt + 2))[:n]
    rng = np.random.default_rng(seed)
    rng.shuffle(s)
    return {lo + i: int(s[i]) for i in range(n)}


def build_program(ns: int):
    q = ns // 4          # free extent of the packed [128, q] layout
    qs = q // NSTR       # columns per stream
    nsl = q // 512       # 512-wide prologue slices
    assert ns % 2048 == 0 and nsl >= 1 and qs % 256 == 0

    nc = bacc.Bacc("TRN2", target_bir_lowering=False)

    n_ramp_tiles = N_RAMP + (N_RAMP - 1)   # w_1..12 then bid_2..12
    x_d = nc.dram_tensor("x", [M, ns], BF16, kind="ExternalInput")
    wramp_d = nc.dram_tensor("wramp", [128, n_ramp_tiles * 128], BF16,
                             kind="ExternalInput")
    ppad_d = nc.dram_tensor("ppad", [128, 16 * 128], BF16,
                            kind="ExternalInput")
    # wd variants then bid_const
    wconst_d = nc.dram_tensor("wconst", [128, (N_DITHER + 1) * 128], BF16,
                              kind="ExternalInput")
    w32_d = nc.dram_tensor("w32", [128, (N_DITHER32 + 1) * 128], F32,
                           kind="ExternalInput")
    out_d = nc.dram_tensor("s_out", [KD, ns], F32, kind="ExternalOutput")

    # dither schedules: bf16 constant phase k=13..25, fp32r phase k=26..32
    sched_b = _sched(N_BF16 + 1 - 13 + 1, 13, N_DITHER, DITHER_SEED)
    sched_f = _sched(TOTAL_UPD - 26 + 1, 26, N_DITHER32, DITHER_SEED + 1)

    with ExitStack() as ctx:
        tc = ctx.enter_context(tile.TileContext(nc))
        persist = ctx.enter_context(tc.tile_pool(name="persist", bufs=1))
        xpool = ctx.enter_context(tc.tile_pool(name="xstage", bufs=1))
        psum = ctx.enter_context(tc.tile_pool(name="psum", bufs=2,
                                              space="PSUM"))

        # ramp weights first on sync (warm-up + early loop need them);
        # X rides both queues right after; late-phase weights trail.
        wramp_sb = persist.tile([128, n_ramp_tiles * 128], BF16)
        nc.sync.dma_start(wramp_sb[:], wramp_d[:])
        pp_sb = persist.tile([128, 16 * 128], BF16)
        nc.scalar.dma_start(pp_sb[:], ppad_d[:])

        def wr(i):          # ramp tile i
            return wramp_sb[:, 128 * i:128 * (i + 1)]

        # PE warm-up on the first ramp tile: junk matmuls release the
        # PE_HAM throttle during the X-DMA window.
        warm = psum.tile([128, 512], F32, name="warm", tag="pt1")
        for wi in range(24):
            nc.tensor.matmul(warm[:, 0:128], wr(0), wr(0),
                             start=(wi == 0), stop=(wi == 23))

        xts = {}
        for c in range(4):
            for h in range(2):
                xt = xpool.tile([128, ns // 2], BF16, name=f"xt{c}{h}")
                (nc.sync if h == 0 else nc.scalar).dma_start(
                    xt[:],
                    x_d[128 * c:128 * (c + 1),
                        (ns // 2) * h:(ns // 2) * (h + 1)])
                xts[(c, h)] = xt

        wconst_sb = persist.tile([128, (N_DITHER + 1) * 128], BF16)
        nc.scalar.dma_start(wconst_sb[:], wconst_d[:])
        w32_sb = persist.tile([128, (N_DITHER32 + 1) * 128], F32R)
        nc.scalar.dma_start(w32_sb[:], w32_d[:].bitcast(F32R))

        e_sb = persist.tile([128, q], F32)
        v_st = [[persist.tile([128, qs], BF16, name=f"v{i}_{j}")
                 for j in range(NSTR)] for i in range(3)]
        v32 = [[persist.tile([128, qs], F32R, name=f"v32_{i}_{j}")
                for j in range(NSTR)] for i in range(3)]
        sout = persist.tile([128, q], F32)

        # ---- prologue: E = (-A(AtA)^-1).T @ X in packed layout ----
        pbs = [psum.tile([128, 512], F32, name=f"pb{s}", tag=f"pt{2 * s}")
               for s in range(nsl)]
        n_acc = [0] * nsl
        for c in range(4):
            for h in range(2):
                xt = xts[(c, h)]
                for g in (2 * h, 2 * h + 1):
                    lhs = pp_sb[:, 128 * (4 * c + g):128 * (4 * c + g + 1)]
                    for s in range(nsl):
                        i_acc = n_acc[s]
                        n_acc[s] = i_acc + 1
                        xoff = (g % 2) * q + 512 * s
                        nc.tensor.matmul(
                            pbs[s][:], lhs, xt[:, xoff:xoff + 512],
                            start=(i_acc == 0), stop=(i_acc == 15))
        spp = 512 // qs  # streams per prologue slice
        for s in range(nsl):
            nc.scalar.copy(e_sb[:, 512 * s:512 * (s + 1)], pbs[s][:])
            for jj in range(spp):
                j = spp * s + jj
                src = pbs[s][:, qs * jj:qs * (jj + 1)]
                # V1 = relu(E) straight from PSUM; Act also has the E
                # copies, so it only takes one stream
                if j == 1:
                    nc.scalar.activation(v_st[1][j][:], src,
                                         mybir.ActivationFunctionType.Relu)
                else:
                    nc.vector.tensor_scalar_max(v_st[1][j][:], src, 0.0)

        # ---- loop: update k computes V_{k+1} from V_k, V_{k-1} ----
        # state dtype: V_j is bf16 for j <= N_BF16+1, f32 after
        def vt(jgen, j):
            return (v_st if jgen <= N_BF16 + 1 else v32)[jgen % 3][j]

        for k in range(1, TOTAL_UPD + 1):
            last = (k == TOTAL_UPD)
            # cur-operand weight flavor (matches dtype of V_k)
            if k <= N_RAMP:
                wcur = wr(k - 1)
            elif k <= N_BF16 + 1:
                wcur = wconst_sb[:, 128 * sched_b[k]:128 * (sched_b[k] + 1)]
            else:
                wcur = w32_sb[:, 128 * sched_f[k]:128 * (sched_f[k] + 1)]
            # prev-operand flavor (matches dtype of V_{k-1}); k=1: b_1=0
            if k == 1:
                wprev = None
            elif k <= N_RAMP:
                wprev = wr(N_RAMP + k - 2)          # bid_k ramp tile
            elif k <= N_BF16 + 2:
                wprev = wconst_sb[:, 128 * N_DITHER:128 * (N_DITHER + 1)]
            else:
                wprev = w32_sb[:, 128 * N_DITHER32:128 * (N_DITHER32 + 1)]

            for j in range(NSTR):
                cur = vt(k, j) if k > 1 else v_st[1][j]
                prev = vt(k - 1, j) if k > 1 else None
                ptb = psum.tile([128, 512], F32, name=f"pt{k}_{j}",
                                tag=f"pt{j}")
                pt = ptb[:, 0:qs]
                esl = e_sb[:, qs * j:qs * (j + 1)]
                if k == 1:
                    nc.tensor.matmul(pt, wcur, cur[:],
                                     start=True, stop=True)
                else:
                    # -b*V_{k-1} first: never waits on the newest state
                    nc.tensor.matmul(pt, wprev, prev[:],
                                     start=True, stop=False)
                    nc.tensor.matmul(pt, wcur, cur[:],
                                     start=False, stop=True)
                if last:
                    # S = max(P,E)-E = relu(P-E): sub in PSUM, relu on Act
                    nc.vector.tensor_tensor(pt, pt, esl,
                                            op=mybir.AluOpType.subtract)
                    nc.scalar.activation(sout[:, qs * j:qs * (j + 1)], pt,
                                         mybir.ActivationFunctionType.Relu)
                else:
                    nc.vector.tensor_tensor(vt(k + 1, j)[:], pt, esl,
                                            op=mybir.AluOpType.max)

        for g in range(4):
            (nc.sync if g % 2 == 0 else nc.scalar).dma_start(
                out_d[:, g * q:(g + 1) * q], sout[32 * g:32 * (g + 1), :])

    nc.finalize()
    return nc


def _bf16(x):
    return np.asarray(x, dtype=np.float32).astype(ml_dtypes.bfloat16)


def _bf16_val(x):
    return float(np.float32(ml_dtypes.bfloat16(np.float32(x))))


def _round11(x):
    u = np.ascontiguousarray(np.asarray(x, dtype=np.float32)).view(np.uint32)
    u = ((u + np.uint32(1 << 11)) >> np.uint32(12)) << np.uint32(12)
    return u.view(np.float32)


def _dither(Mx, n, nbits):
    """n reduced-precision matrices whose per-entry mean ~= Mx."""
    M64 = np.asarray(Mx, dtype=np.float64)
    hi = (_bf16(M64) if nbits == 8 else _round11(M64)).astype(np.float64)
    ulp = 2.0 ** (np.floor(np.log2(np.abs(M64) + 1e-300)) - nbits)
    flo = np.where(hi > M64, hi - ulp, hi)
    fhi = flo + ulp
    frac = np.clip((M64 - flo) / ulp, 0, 1)
    cnt = np.rint(frac * n).astype(int)
    return [np.where(i < cnt, fhi, flo).astype(np.float32) for i in range(n)]


def _blockdiag(Mt, dtype):
    """lhsT tile: 4-group block-diagonal of Mt (already transposed)."""
    out = np.zeros((128, 128), dtype=dtype)
    for g in range(4):
        out[32 * g:32 * (g + 1), 32 * g:32 * (g + 1)] = Mt
    return out


def _cheby_params(L, mu, n_ramp, n_total):
    d = (L + mu) / 2.0
    cc = (L - mu) / 2.0
    al = [0.0] * (n_total + 1)
    be = [0.0] * (n_total + 1)
    w_prev = 0.0
    for k in range(1, n_total + 1):
        w = 1.0 / d if k == 1 else 1.0 / (d - cc * cc / 4.0 * w_prev)
        be[k] = (cc / 2.0) ** 2 * w_prev * w if k > 1 else 0.0
        al[k] = w
        w_prev = w
    kap = L / mu
    aH = ALPHA_BACKOFF * 4.0 / (np.sqrt(L) + np.sqrt(mu)) ** 2
    bH = ((np.sqrt(kap) - 1.0) / (np.sqrt(kap) + 1.0)) ** 2
    for k in range(n_ramp + 1, n_total + 1):
        al[k] = aH
        be[k] = bH
    return al, be


def host_prep(A: np.ndarray):
    A64 = np.asarray(A, dtype=np.float64)
    AtA = A64.T @ A64
    ev = np.linalg.eigvalsh(AtA)
    L, mu = ev[-1], ev[0]
    I = np.eye(KD)
    al, be = _cheby_params(L, mu, N_RAMP, TOTAL_UPD)

    # ramp tiles: w_1..N_RAMP (whole Wa_k, nearest-bf16), bid_2..N_RAMP
    n_ramp_tiles = N_RAMP + (N_RAMP - 1)
    wramp = np.zeros((128, n_ramp_tiles * 128), dtype=ml_dtypes.bfloat16)
    for k in range(1, N_RAMP + 1):
        bq = _bf16_val(be[k])
        Wa = (1.0 + bq) * I - al[k] * AtA
        wramp[:, 128 * (k - 1):128 * k] = _blockdiag(
            _bf16(Wa.T), ml_dtypes.bfloat16)
        if k >= 2:
            bid = np.zeros((32, 32), dtype=np.float64)
            np.fill_diagonal(bid, -bq)
            wramp[:, 128 * (N_RAMP + k - 2):128 * (N_RAMP + k - 1)] = \
                _blockdiag(_bf16(bid), ml_dtypes.bfloat16)

    # constant phase: aH, bH
    aH, bH = al[N_RAMP + 1], be[N_RAMP + 1]
    bHq = _bf16_val(bH)
    WaH = (1.0 + bHq) * I - aH * AtA
    wconst = np.zeros((128, (N_DITHER + 1) * 128), dtype=ml_dtypes.bfloat16)
    for i, v in enumerate(_dither(WaH.T, N_DITHER, 8)):
        wconst[:, 128 * i:128 * (i + 1)] = _blockdiag(v, ml_dtypes.bfloat16)
    bidH = np.diag([-bHq] * KD)
    wconst[:, 128 * N_DITHER:] = _blockdiag(
        _bf16(bidH), ml_dtypes.bfloat16)

    # fp32r phase (round11-dithered), exact-f32 beta diag
    bH32 = float(np.float32(bH))
    WaH32 = (1.0 + bH32) * I - aH * AtA
    w32 = np.zeros((128, (N_DITHER32 + 1) * 128), dtype=np.float32)
    for i, v in enumerate(_dither(WaH32.T, N_DITHER32, 11)):
        w32[:, 128 * i:128 * (i + 1)] = _blockdiag(v, np.float32)
    w32[:, 128 * N_DITHER32:] = _blockdiag(
        np.diag([-bH32] * KD).astype(np.float32), np.float32)

    # prologue lhs: -A(AtA)^-1 row-chunks, per-chunk dithered bf16
    P = A64 @ np.linalg.inv(AtA)
    ppad = np.zeros((128, 16 * 128), dtype=ml_dtypes.bfloat16)
    for cch in range(4):
        pv = _dither(-P[128 * cch:128 * (cch + 1), :], 4, 8)[cch % 4]
        for g in range(4):
            ppad[:, 128 * (4 * cch + g) + 32 * g:
                 128 * (4 * cch + g) + 32 * (g + 1)] = _bf16(pv)
    return wramp, ppad, wconst, w32


_PROGRAM_CACHE = {}


def _get_program(ns):
    if ns not in _PROGRAM_CACHE:
        _PROGRAM_CACHE[ns] = build_program(ns)
    return _PROGRAM_CACHE[ns]


def kernel(X: np.ndarray, A: np.ndarray) -> np.ndarray:
    global LAST_RESULTS
    X = np.ascontiguousarray(np.asarray(X, dtype=np.float32))
    A = np.ascontiguousarray(np.asarray(A, dtype=np.float32))
    assert X.shape == (M, N_FULL) and A.shape == (M, KD)

    ns = N_FULL // N_CORES
    wramp, ppad, wconst, w32 = host_prep(A)
    nc = _get_program(ns)

    Xb = _bf16(X)
    in_maps = []
    for c in range(N_CORES):
        in_maps.append({
            "x": np.ascontiguousarray(Xb[:, c * ns:(c + 1) * ns]),
            "wramp": wramp,
            "ppad": ppad,
            "wconst": wconst,
            "w32": w32,
        })

    res = run_bass_kernel_spmd(nc, in_maps, core_ids=list(range(N_CORES)))
    LAST_RESULTS = res
    S = np.concatenate([res.results[c]["s_out"] for c in range(N_CORES)],
                       axis=1)
    return np.ascontiguousarray(S.astype(np.float32))


# revision 11
# speedup vs baseline: 1.9594x; 1.1708x over previous
"""Trainium2 Bass kernel for nn_LsqNonneg: batched NNLS.

Algorithm: projected Chebyshev/heavy-ball on the V-substitution of the NNLS
KKT iteration.  With Wa_k = (1+b_k)I - a_k*AtA, the S-iteration

    S_{k+1} = relu(Wa_k S_k - b_k S_{k-1} + a_k AtX)

becomes, under V := S - Sunc with Sunc = (AtA)^-1 AtX  (E := -Sunc):

    V_{k+1} = max(Wa_k V_k - b_k V_{k-1}, E),     S = V - E

i.e. the per-iteration bias add AND the relu collapse into a single DVE
tensor_tensor(max) against the fixed threshold tile E (computed in the
prologue as (-A(AtA)^-1).T @ X -- same cost as computing AtX).

Schedule: 12 Chebyshev ramp iterations (exact [mu, L] from the host
eigendecomposition of AtA) then constant heavy-ball at the optimum
(alpha backed off 5% from the lambda=L stability edge).  Chebyshev's
transient reaches the same error ~12 iterations earlier than constant
momentum from the warm start V_1 = relu(E).

Precision: phase 1 (24 updates) runs bf16 weights+states -- its ~1e-2
bf16-state-rounding floor is then crushed by phase 2: 8 fp32r polish
updates on f32 states (fp32r noise ~2^-12), contracting the bf16-phase
noise by rho^8 while the weights carry 11-bit-dithered variants.  E stays
f32 throughout (it biases the fixed point 1:1).  The phase transition is
seamless: each matmul picks the weight flavor matching its state operand's
dtype, so V_25 (bf16) and V_26 (f32) coexist inside one update.

All weight tensors are host-packed into exact SBUF layouts (contiguous
per-partition DMAs); X rides both HW DMA queues (SP+Act) right behind the
small ramp weights, and the late-phase weights trail the X chunks since
they are not needed until ~35us in.
"""

import os
import sys

import numpy as np

for _p in ("/opt/trn_rl_repo", "/root/.axon_site/_ro/trn_rl_repo"):
    if os.path.isdir(_p) and _p not in sys.path:
        sys.path.append(_p)

import ml_dtypes
from contextlib import ExitStack

import concourse.bass as bass
import concourse.bacc as bacc
import concourse.tile as tile
from concourse import mybir
from concourse.bass_utils import run_bass_kernel_spmd

M, KD, N_FULL, N_CORES = 512, 32, 32768, 8
N_RAMP = 12          # Chebyshev ramp updates (per-iteration weights)
N_BF16 = 24          # total bf16-phase updates (incl. ramp)
N_POLISH = 8         # fp32r polish updates
TOTAL_UPD = N_BF16 + N_POLISH
N_DITHER = 8         # bf16 constant-phase dither variants
N_DITHER32 = 4       # fp32r-phase dither variants
DITHER_SEED = 1
NSTR = 4             # independent column streams in the loop
ALPHA_BACKOFF = 0.95

F32 = mybir.dt.float32
F32R = mybir.dt.float32r
BF16 = mybir.dt.bfloat16

LAST_RESULTS = None  # BassKernelResults of the most recent run (for test.py)


def _sched(n, lo, cnt, seed):
    s = np.concatenate([np.arange(cnt)] * (n // cnt + 2))[:n]
    rng = np.random.default_rng(seed)
    rng.shuffle(s)
    return {lo + i: int(s[i]) for i in range(n)}


def build_program(ns: int):
    q = ns // 4          # free extent of the packed [128, q] layout
    qs = q // NSTR       # columns per stream
    nsl = q // 512       # 512-wide prologue slices
    assert ns % 2048 == 0 and nsl >= 1 and qs % 256 == 0

    nc = bacc.Bacc("TRN2", target_bir_lowering=False)

    n_ramp_tiles = N_RAMP + (N_RAMP - 1)   # w_1..12 then bid_2..12
    x_d = nc.dram_tensor("x", [M, ns], BF16, kind="ExternalInput")
    wr1_d = nc.dram_tensor("wr1", [128, 128], BF16, kind="ExternalInput")
    wramp_d = nc.dram_tensor("wramp", [128, n_ramp_tiles * 128], BF16,
                             kind="ExternalInput")
    ppad_d = nc.dram_tensor("ppad", [128, 16 * 128], BF16,
                            kind="ExternalInput")
    # wd variants then bid_const
    wconst_d = nc.dram_tensor("wconst", [128, (N_DITHER + 1) * 128], BF16,
                              kind="ExternalInput")
    w32_d = nc.dram_tensor("w32", [128, (N_DITHER32 + 1) * 128], F32,
                           kind="ExternalInput")
    out_d = nc.dram_tensor("s_out", [KD, ns], F32, kind="ExternalOutput")

    # dither schedules: bf16 constant phase k=13..25, fp32r phase k=26..32
    sched_b = _sched(N_BF16 + 1 - 13 + 1, 13, N_DITHER, DITHER_SEED)
    sched_f = _sched(TOTAL_UPD - 26 + 1, 26, N_DITHER32, DITHER_SEED + 1)

    with ExitStack() as ctx:
        tc = ctx.enter_context(tile.TileContext(nc))
        persist = ctx.enter_context(tc.tile_pool(name="persist", bufs=1))
        xpool = ctx.enter_context(tc.tile_pool(name="xstage", bufs=1))
        psum = ctx.enter_context(tc.tile_pool(name="psum", bufs=2,
                                              space="PSUM"))

        # w_1 (tiny) leads sync so update 1 never waits; ppad leads the
        # scalar ring for the warm-up + prologue; X rides both queues
        # next; the bulk ramp/late-phase weights trail the X chunks.
        wr1_sb = persist.tile([128, 128], BF16)
        nc.sync.dma_start(wr1_sb[:], wr1_d[:])
        pp_sb = persist.tile([128, 16 * 128], BF16)
        nc.scalar.dma_start(pp_sb[:], ppad_d[:])

        # PE warm-up on ppad (earliest-arriving tile): junk matmuls
        # release the PE_HAM throttle during the X-DMA window.
        warm = psum.tile([128, 512], F32, name="warm", tag="pt1")
        for wi in range(24):
            nc.tensor.matmul(warm[:, 0:128], pp_sb[:, 0:128],
                             pp_sb[:, 0:128],
                             start=(wi == 0), stop=(wi == 23))

        xts = {}
        for c in range(4):
            for h in range(2):
                xt = xpool.tile([128, ns // 2], BF16, name=f"xt{c}{h}")
                (nc.sync if h == 0 else nc.scalar).dma_start(
                    xt[:],
                    x_d[128 * c:128 * (c + 1),
                        (ns // 2) * h:(ns // 2) * (h + 1)])
                xts[(c, h)] = xt

        wramp_sb = persist.tile([128, n_ramp_tiles * 128], BF16)
        nc.sync.dma_start(wramp_sb[:], wramp_d[:])

        def wr(i):          # ramp tile i (update 1 uses the early copy)
            return wr1_sb[:] if i == 0 else wramp_sb[:, 128 * i:128 * (i + 1)]

        wconst_sb = persist.tile([128, (N_DITHER + 1) * 128], BF16)
        nc.scalar.dma_start(wconst_sb[:], wconst_d[:])
        w32_sb = persist.tile([128, (N_DITHER32 + 1) * 128], F32R)
        nc.scalar.dma_start(w32_sb[:], w32_d[:].bitcast(F32R))

        e_sb = persist.tile([128, q], F32)
        v_st = [[persist.tile([128, qs], BF16, name=f"v{i}_{j}")
                 for j in range(NSTR)] for i in range(3)]
        v32 = [[persist.tile([128, qs], F32R, name=f"v32_{i}_{j}")
                for j in range(NSTR)] for i in range(3)]
        sout = persist.tile([128, q], F32)

        # ---- prologue: E = (-A(AtA)^-1).T @ X in packed layout ----
        pbs = [psum.tile([128, 512], F32, name=f"pb{s}", tag=f"pt{2 * s}")
               for s in range(nsl)]
        n_acc = [0] * nsl
        for c in range(4):
            for h in range(2):
                xt = xts[(c, h)]
                for g in (2 * h, 2 * h + 1):
                    lhs = pp_sb[:, 128 * (4 * c + g):128 * (4 * c + g + 1)]
                    for s in range(nsl):
                        i_acc = n_acc[s]
                        n_acc[s] = i_acc + 1
                        xoff = (g % 2) * q + 512 * s
                        nc.tensor.matmul(
                            pbs[s][:], lhs, xt[:, xoff:xoff + 512],
                            start=(i_acc == 0), stop=(i_acc == 15))
        spp = 512 // qs  # streams per prologue slice
        for s in range(nsl):
            nc.scalar.copy(e_sb[:, 512 * s:512 * (s + 1)], pbs[s][:])
            for jj in range(spp):
                j = spp * s + jj
                src = pbs[s][:, qs * jj:qs * (jj + 1)]
                # V1 = relu(E) straight from PSUM; Act also has the E
                # copies, so it only takes one stream
                if j == 1:
                    nc.scalar.activation(v_st[1][j][:], src,
                                         mybir.ActivationFunctionType.Relu)
                else:
                    nc.vector.tensor_scalar_max(v_st[1][j][:], src, 0.0)

        # ---- loop: update k computes V_{k+1} from V_k, V_{k-1} ----
        # state dtype: V_j is bf16 for j <= N_BF16+1, f32 after
        def vt(jgen, j):
            return (v_st if jgen <= N_BF16 + 1 else v32)[jgen % 3][j]

        for k in range(1, TOTAL_UPD + 1):
            last = (k == TOTAL_UPD)
            # cur-operand weight flavor (matches dtype of V_k)
            if k <= N_RAMP:
                wcur = wr(k - 1)
            elif k <= N_BF16 + 1:
                wcur = wconst_sb[:, 128 * sched_b[k]:128 * (sched_b[k] + 1)]
            else:
                wcur = w32_sb[:, 128 * sched_f[k]:128 * (sched_f[k] + 1)]
            # prev-operand flavor (matches dtype of V_{k-1}); k=1: b_1=0
            if k == 1:
                wprev = None
            elif k <= N_RAMP:
                wprev = wr(N_RAMP + k - 2)          # bid_k ramp tile
            elif k <= N_BF16 + 2:
                wprev = wconst_sb[:, 128 * N_DITHER:128 * (N_DITHER + 1)]
            else:
                wprev = w32_sb[:, 128 * N_DITHER32:128 * (N_DITHER32 + 1)]

            for j in range(NSTR):
                cur = vt(k, j) if k > 1 else v_st[1][j]
                prev = vt(k - 1, j) if k > 1 else None
                ptb = psum.tile([128, 512], F32, name=f"pt{k}_{j}",
                                tag=f"pt{j}")
                pt = ptb[:, 0:qs]
                esl = e_sb[:, qs * j:qs * (j + 1)]
                if k == 1:
                    nc.tensor.matmul(pt, wcur, cur[:],
                                     start=True, stop=True)
                else:
                    # -b*V_{k-1} first: never waits on the newest state
                    nc.tensor.matmul(pt, wprev, prev[:],
                                     start=True, stop=False)
                    nc.tensor.matmul(pt, wcur, cur[:],
                                     start=False, stop=True)
                if last:
                    # S = max(P,E)-E = relu(P-E): sub in PSUM, relu on Act
                    nc.vector.tensor_tensor(pt, pt, esl,
                                            op=mybir.AluOpType.subtract)
                    nc.scalar.activation(sout[:, qs * j:qs * (j + 1)], pt,
                                         mybir.ActivationFunctionType.Relu)
                else:
                    nc.vector.tensor_tensor(vt(k + 1, j)[:], pt, esl,
                                            op=mybir.AluOpType.max)

        for g in range(4):
            (nc.sync if g % 2 == 0 else nc.scalar).dma_start(
                out_d[:, g * q:(g + 1) * q], sout[32 * g:32 * (g + 1), :])

    nc.finalize()
    return nc


def _bf16(x):
    return np.asarray(x, dtype=np.float32).astype(ml_dtypes.bfloat16)


def _bf16_val(x):
    return float(np.float32(ml_dtypes.bfloat16(np.float32(x))))


def _round11(x):
    u = np.ascontiguousarray(np.asarray(x, dtype=np.float32)).view(np.uint32)
    u = ((u + np.uint32(1 << 11)) >> np.uint32(12)) << np.uint32(12)
    return u.view(np.float32)


def _dither(Mx, n, nbits):
    """n reduced-precision matrices whose per-entry mean ~= Mx."""
    M64 = np.asarray(Mx, dtype=np.float64)
    hi = (_bf16(M64) if nbits == 8 else _round11(M64)).astype(np.float64)
    ulp = 2.0 ** (np.floor(np.log2(np.abs(M64) + 1e-300)) - nbits)
    flo = np.where(hi > M64, hi - ulp, hi)
    fhi = flo + ulp
    frac = np.clip((M64 - flo) / ulp, 0, 1)
    cnt = np.rint(frac * n).astype(int)
    return [np.where(i < cnt, fhi, flo).astype(np.float32) for i in range(n)]


def _blockdiag(Mt, dtype):
    """lhsT tile: 4-group block-diagonal of Mt (already transposed)."""
    out = np.zeros((128, 128), dtype=dtype)
    for g in range(4):
        out[32 * g:32 * (g + 1), 32 * g:32 * (g + 1)] = Mt
    return out


def _cheby_params(L, mu, n_ramp, n_total):
    d = (L + mu) / 2.0
    cc = (L - mu) / 2.0
    al = [0.0] * (n_total + 1)
    be = [0.0] * (n_total + 1)
    w_prev = 0.0
    for k in range(1, n_total + 1):
        w = 1.0 / d if k == 1 else 1.0 / (d - cc * cc / 4.0 * w_prev)
        be[k] = (cc / 2.0) ** 2 * w_prev * w if k > 1 else 0.0
        al[k] = w
        w_prev = w
    kap = L / mu
    aH = ALPHA_BACKOFF * 4.0 / (np.sqrt(L) + np.sqrt(mu)) ** 2
    bH = ((np.sqrt(kap) - 1.0) / (np.sqrt(kap) + 1.0)) ** 2
    for k in range(n_ramp + 1, n_total + 1):
        al[k] = aH
        be[k] = bH
    return al, be


def host_prep(A: np.ndarray):
    A64 = np.asarray(A, dtype=np.float64)
    AtA = A64.T @ A64
    ev = np.linalg.eigvalsh(AtA)
    L, mu = ev[-1], ev[0]
    I = np.eye(KD)
    al, be = _cheby_params(L, mu, N_RAMP, TOTAL_UPD)

    # ramp tiles: w_1..N_RAMP (whole Wa_k, nearest-bf16), bid_2..N_RAMP
    n_ramp_tiles = N_RAMP + (N_RAMP - 1)
    wramp = np.zeros((128, n_ramp_tiles * 128), dtype=ml_dtypes.bfloat16)
    for k in range(1, N_RAMP + 1):
        bq = _bf16_val(be[k])
        Wa = (1.0 + bq) * I - al[k] * AtA
        wramp[:, 128 * (k - 1):128 * k] = _blockdiag(
            _bf16(Wa.T), ml_dtypes.bfloat16)
        if k >= 2:
            bid = np.zeros((32, 32), dtype=np.float64)
            np.fill_diagonal(bid, -bq)
            wramp[:, 128 * (N_RAMP + k - 2):128 * (N_RAMP + k - 1)] = \
                _blockdiag(_bf16(bid), ml_dtypes.bfloat16)

    # constant phase: aH, bH
    aH, bH = al[N_RAMP + 1], be[N_RAMP + 1]
    bHq = _bf16_val(bH)
    WaH = (1.0 + bHq) * I - aH * AtA
    wconst = np.zeros((128, (N_DITHER + 1) * 128), dtype=ml_dtypes.bfloat16)
    for i, v in enumerate(_dither(WaH.T, N_DITHER, 8)):
        wconst[:, 128 * i:128 * (i + 1)] = _blockdiag(v, ml_dtypes.bfloat16)
    bidH = np.diag([-bHq] * KD)
    wconst[:, 128 * N_DITHER:] = _blockdiag(
        _bf16(bidH), ml_dtypes.bfloat16)

    # fp32r phase (round11-dithered), exact-f32 beta diag
    bH32 = float(np.float32(bH))
    WaH32 = (1.0 + bH32) * I - aH * AtA
    w32 = np.zeros((128, (N_DITHER32 + 1) * 128), dtype=np.float32)
    for i, v in enumerate(_dither(WaH32.T, N_DITHER32, 11)):
        w32[:, 128 * i:128 * (i + 1)] = _blockdiag(v, np.float32)
    w32[:, 128 * N_DITHER32:] = _blockdiag(
        np.diag([-bH32] * KD).astype(np.float32), np.float32)

    # prologue lhs: -A(AtA)^-1 row-chunks, per-chunk dithered bf16
    P = A64 @ np.linalg.inv(AtA)
    ppad = np.zeros((128, 16 * 128), dtype=ml_dtypes.bfloat16)
    for cch in range(4):
        pv = _dither(-P[128 * cch:128 * (cch + 1), :], 4, 8)[cch % 4]
        for g in range(4):
            ppad[:, 128 * (4 * cch + g) + 32 * g:
                 128 * (4 * cch + g) + 32 * (g + 1)] = _bf16(pv)
    return wramp, ppad, wconst, w32


_PROGRAM_CACHE = {}


def _get_program(ns):
    if ns not in _PROGRAM_CACHE:
        _PROGRAM_CACHE[ns] = build_program(ns)
    return _PROGRAM_CACHE[ns]


def kernel(X: np.ndarray, A: np.ndarray) -> np.ndarray:
    global LAST_RESULTS
    X = np.ascontiguousarray(np.asarray(X, dtype=np.float32))
    A = np.ascontiguousarray(np.asarray(A, dtype=np.float32))
    assert X.shape == (M, N_FULL) and A.shape == (M, KD)

    ns = N_FULL // N_CORES
    wramp, ppad, wconst, w32 = host_prep(A)
    nc = _get_program(ns)

    Xb = _bf16(X)
    wr1 = np.ascontiguousarray(wramp[:, 0:128])
    in_maps = []
    for c in range(N_CORES):
        in_maps.append({
            "x": np.ascontiguousarray(Xb[:, c * ns:(c + 1) * ns]),
            "wr1": wr1,
            "wramp": wramp,
            "ppad": ppad,
            "wconst": wconst,
            "w32": w32,
        })

    res = run_bass_kernel_spmd(nc, in_maps, core_ids=list(range(N_CORES)))
    LAST_RESULTS = res
    S = np.concatenate([res.results[c]["s_out"] for c in range(N_CORES)],
                       axis=1)
    return np.ascontiguousarray(S.astype(np.float32))


# revision 30
# speedup vs baseline: 2.0470x; 1.0447x over previous
"""Trainium2 Bass kernel for nn_LsqNonneg: batched NNLS.

Algorithm: projected Chebyshev/heavy-ball on the V-substitution of the NNLS
KKT iteration.  With Wa_k = (1+b_k)I - a_k*AtA, the S-iteration

    S_{k+1} = relu(Wa_k S_k - b_k S_{k-1} + a_k AtX)

becomes, under V := S - Sunc with Sunc = (AtA)^-1 AtX  (E := -Sunc):

    V_{k+1} = max(Wa_k V_k - b_k V_{k-1}, E),     S = V - E

i.e. the per-iteration bias add AND the relu collapse into a single DVE
tensor_tensor(max) against the fixed threshold tile E (computed in the
prologue as (-A(AtA)^-1).T @ X -- same cost as computing AtX).

Schedule: 12 Chebyshev ramp iterations (exact [mu, L] from the host
eigendecomposition of AtA) then constant heavy-ball at the optimum
(alpha backed off 5% from the lambda=L stability edge).  Chebyshev's
transient reaches the same error ~12 iterations earlier than constant
momentum from the warm start V_1 = relu(E).

Precision: phase 1 (24 updates) runs bf16 weights+states -- its ~1e-2
bf16-state-rounding floor is then crushed by phase 2: 8 fp32r polish
updates on f32 states (fp32r noise ~2^-12), contracting the bf16-phase
noise by rho^8 while the weights carry 11-bit-dithered variants.  E stays
f32 throughout (it biases the fixed point 1:1).  The phase transition is
seamless: each matmul picks the weight flavor matching its state operand's
dtype, so V_25 (bf16) and V_26 (f32) coexist inside one update.

All weight tensors are host-packed into exact SBUF layouts (contiguous
per-partition DMAs); X rides both HW DMA queues (SP+Act) right behind the
small ramp weights, and the late-phase weights trail the X chunks since
they are not needed until ~35us in.
"""

import os
import sys

import numpy as np

for _p in ("/opt/trn_rl_repo", "/root/.axon_site/_ro/trn_rl_repo"):
    if os.path.isdir(_p) and _p not in sys.path:
        sys.path.append(_p)

import ml_dtypes
from contextlib import ExitStack

import concourse.bass as bass
import concourse.bacc as bacc
import concourse.tile as tile
from concourse import mybir
from concourse.bass_utils import run_bass_kernel_spmd

M, KD, N_FULL, N_CORES = 512, 32, 32768, 8
N_RAMP = 12          # Chebyshev ramp updates (per-iteration weights)
N_BF16 = 20          # total bf16-phase updates (incl. ramp)
N_POLISH = 6         # fp32r polish updates
TOTAL_UPD = N_BF16 + N_POLISH
N_CONST = N_BF16 + 1 - N_RAMP   # constant-phase bf16 updates (k=13..21)
SD_RHO = 0.856       # heavy-ball contraction: sigma-delta discount factor
NSTR = 4             # independent column streams in the loop
ALPHA_BACKOFF = 0.95

F32 = mybir.dt.float32
F32R = mybir.dt.float32r
BF16 = mybir.dt.bfloat16

LAST_RESULTS = None  # BassKernelResults of the most recent run (for test.py)


def build_program(ns: int):
    q = ns // 4          # free extent of the packed [128, q] layout
    qs = q // NSTR       # columns per stream
    nsl = q // 512       # 512-wide prologue slices
    assert ns % 2048 == 0 and nsl >= 1 and qs % 256 == 0

    nc = bacc.Bacc("TRN2", target_bir_lowering=False)

    n_ramp_tiles = N_RAMP + (N_RAMP - 1)   # w_1..12 then bid_2..12
    x_d = nc.dram_tensor("x", [M, ns], BF16, kind="ExternalInput")
    wr1_d = nc.dram_tensor("wr1", [128, 128], BF16, kind="ExternalInput")
    wramp_d = nc.dram_tensor("wramp", [128, n_ramp_tiles * 128], BF16,
                             kind="ExternalInput")
    ppad_d = nc.dram_tensor("ppad", [128, 16 * 128], BF16,
                            kind="ExternalInput")
    # sigma-delta per-update tiles then bid_const
    wconst_d = nc.dram_tensor("wconst", [128, (N_CONST + 1) * 128], BF16,
                              kind="ExternalInput")
    n_w32 = TOTAL_UPD - (N_BF16 + 1)       # updates whose cur-state is f32
    w32_d = nc.dram_tensor("w32", [128, (n_w32 + 1) * 128], F32,
                           kind="ExternalInput")
    out_d = nc.dram_tensor("s_out", [KD, ns], F32, kind="ExternalOutput")

    with ExitStack() as ctx:
        tc = ctx.enter_context(tile.TileContext(nc))
        persist = ctx.enter_context(tc.tile_pool(name="persist", bufs=1))
        xpool = ctx.enter_context(tc.tile_pool(name="xstage", bufs=1))
        psum = ctx.enter_context(tc.tile_pool(name="psum", bufs=2,
                                              space="PSUM"))

        # w_1 (tiny) leads sync so update 1 never waits; ppad leads the
        # scalar ring for the warm-up + prologue; X rides both queues
        # next; the bulk ramp/late-phase weights trail the X chunks.
        wr1_sb = persist.tile([128, 128], BF16)
        nc.sync.dma_start(wr1_sb[:], wr1_d[:])
        pp_sb = persist.tile([128, 16 * 128], BF16)
        nc.scalar.dma_start(pp_sb[:], ppad_d[:])

        # PE warm-up on ppad (earliest-arriving tile): junk matmuls
        # release the PE_HAM throttle during the X-DMA window.
        warm = psum.tile([128, 512], F32, name="warm", tag="pt1")
        for wi in range(24):
            nc.tensor.matmul(warm[:, 0:128], pp_sb[:, 0:128],
                             pp_sb[:, 0:128],
                             start=(wi == 0), stop=(wi == 23))

        xts = {}
        for c in range(4):
            for h in range(2):
                xt = xpool.tile([128, ns // 2], BF16, name=f"xt{c}{h}")
                (nc.sync if h == 0 else nc.scalar).dma_start(
                    xt[:],
                    x_d[128 * c:128 * (c + 1),
                        (ns // 2) * h:(ns // 2) * (h + 1)])
                xts[(c, h)] = xt

        wramp_sb = persist.tile([128, n_ramp_tiles * 128], BF16)
        nc.sync.dma_start(wramp_sb[:], wramp_d[:])

        def wr(i):          # ramp tile i (update 1 uses the early copy)
            return wr1_sb[:] if i == 0 else wramp_sb[:, 128 * i:128 * (i + 1)]

        wconst_sb = persist.tile([128, (N_CONST + 1) * 128], BF16)
        nc.scalar.dma_start(wconst_sb[:], wconst_d[:])
        w32_sb = persist.tile([128, (n_w32 + 1) * 128], F32R)
        nc.scalar.dma_start(w32_sb[:], w32_d[:].bitcast(F32R))

        e_sb = persist.tile([128, q], F32)
        v_st = [[persist.tile([128, qs], BF16, name=f"v{i}_{j}")
                 for j in range(NSTR)] for i in range(3)]
        v32 = [[persist.tile([128, qs], F32R, name=f"v32_{i}_{j}")
                for j in range(NSTR)] for i in range(3)]
        sout = persist.tile([128, q], F32)

        # ---- prologue: E = (-A(AtA)^-1).T @ X in packed layout ----
        pbs = [psum.tile([128, 512], F32, name=f"pb{s}", tag=f"pt{2 * s}")
               for s in range(nsl)]
        n_acc = [0] * nsl
        for c in range(4):
            for h in range(2):
                xt = xts[(c, h)]
                for g in (2 * h, 2 * h + 1):
                    lhs = pp_sb[:, 128 * (4 * c + g):128 * (4 * c + g + 1)]
                    for s in range(nsl):
                        i_acc = n_acc[s]
                        n_acc[s] = i_acc + 1
                        xoff = (g % 2) * q + 512 * s
                        nc.tensor.matmul(
                            pbs[s][:], lhs, xt[:, xoff:xoff + 512],
                            start=(i_acc == 0), stop=(i_acc == 15))
        spp = 512 // qs  # streams per prologue slice
        for s in range(nsl):
            nc.scalar.copy(e_sb[:, 512 * s:512 * (s + 1)], pbs[s][:])
            for jj in range(spp):
                j = spp * s + jj
                src = pbs[s][:, qs * jj:qs * (jj + 1)]
                # V1 = relu(E) straight from PSUM; Act also has the E
                # copies, so it only takes one stream
                if j == 1:
                    nc.scalar.activation(v_st[1][j][:], src,
                                         mybir.ActivationFunctionType.Relu)
                else:
                    nc.vector.tensor_scalar_max(v_st[1][j][:], src, 0.0)

        # ---- loop: update k computes V_{k+1} from V_k, V_{k-1} ----
        # state dtype: V_j is bf16 for j <= N_BF16+1, f32 after
        def vt(jgen, j):
            return (v_st if jgen <= N_BF16 + 1 else v32)[jgen % 3][j]

        for k in range(1, TOTAL_UPD + 1):
            last = (k == TOTAL_UPD)
            # cur-operand weight flavor (matches dtype of V_k)
            if k <= N_RAMP:
                wcur = wr(k - 1)
            elif k <= N_BF16 + 1:
                i = k - (N_RAMP + 1)
                wcur = wconst_sb[:, 128 * i:128 * (i + 1)]
            else:
                i = k - (N_BF16 + 2)
                wcur = w32_sb[:, 128 * i:128 * (i + 1)]
            # prev-operand flavor (matches dtype of V_{k-1}); k=1: b_1=0
            if k == 1:
                wprev = None
            elif k <= N_RAMP:
                wprev = wr(N_RAMP + k - 2)          # bid_k ramp tile
            elif k <= N_BF16 + 2:
                wprev = wconst_sb[:, 128 * N_CONST:128 * (N_CONST + 1)]
            else:
                wprev = w32_sb[:, 128 * n_w32:128 * (n_w32 + 1)]

            for j in range(NSTR):
                cur = vt(k, j) if k > 1 else v_st[1][j]
                prev = vt(k - 1, j) if k > 1 else None
                ptb = psum.tile([128, 512], F32, name=f"pt{k}_{j}",
                                tag=f"pt{j}")
                pt = ptb[:, 0:qs]
                esl = e_sb[:, qs * j:qs * (j + 1)]
                if k == 1:
                    nc.tensor.matmul(pt, wcur, cur[:],
                                     start=True, stop=True)
                else:
                    # -b*V_{k-1} first: never waits on the newest state
                    nc.tensor.matmul(pt, wprev, prev[:],
                                     start=True, stop=False)
                    nc.tensor.matmul(pt, wcur, cur[:],
                                     start=False, stop=True)
                if last:
                    # S = max(P,E)-E = relu(P-E): sub in PSUM, relu on Act
                    nc.vector.tensor_tensor(pt, pt, esl,
                                            op=mybir.AluOpType.subtract)
                    nc.scalar.activation(sout[:, qs * j:qs * (j + 1)], pt,
                                         mybir.ActivationFunctionType.Relu)
                else:
                    nc.vector.tensor_tensor(vt(k + 1, j)[:], pt, esl,
                                            op=mybir.AluOpType.max)

        for g in range(4):
            (nc.sync if g % 2 == 0 else nc.scalar).dma_start(
                out_d[:, g * q:(g + 1) * q], sout[32 * g:32 * (g + 1), :])

    nc.finalize()
    return nc


def _bf16(x):
    return np.asarray(x, dtype=np.float32).astype(ml_dtypes.bfloat16)


def _bf16_val(x):
    return float(np.float32(ml_dtypes.bfloat16(np.float32(x))))


def _round11(x):
    u = np.ascontiguousarray(np.asarray(x, dtype=np.float32)).view(np.uint32)
    u = ((u + np.uint32(1 << 11)) >> np.uint32(12)) << np.uint32(12)
    return u.view(np.float32)


def _dither(Mx, n, nbits):
    """n reduced-precision matrices whose per-entry mean ~= Mx."""
    M64 = np.asarray(Mx, dtype=np.float64)
    hi = (_bf16(M64) if nbits == 8 else _round11(M64)).astype(np.float64)
    ulp = 2.0 ** (np.floor(np.log2(np.abs(M64) + 1e-300)) - nbits)
    flo = np.where(hi > M64, hi - ulp, hi)
    fhi = flo + ulp
    frac = np.clip((M64 - flo) / ulp, 0, 1)
    cnt = np.rint(frac * n).astype(int)
    return [np.where(i < cnt, fhi, flo).astype(np.float32) for i in range(n)]


def _grid(M64, nbits):
    """neighboring representable values; nbits = EXPLICIT mantissa bits
    (bf16: 7, fp32r: 11) so the grid matches the storage dtype exactly."""
    hi = (_bf16(M64) if nbits == 7 else _round11(M64)).astype(np.float64)
    ulp = 2.0 ** (np.floor(np.log2(np.abs(M64) + 1e-300)) - nbits)
    flo = np.where(hi > M64, hi - ulp, hi)
    return flo, flo + ulp


def _sigma_delta(Mx, n, rho, nbits):
    """n rounded copies of Mx whose rho-discounted average is unbiased:
    per-entry error-feedback (sigma-delta) choice between the two
    neighboring representable values."""
    M64 = np.asarray(Mx, dtype=np.float64)
    flo, fhi = _grid(M64, nbits)
    D = np.zeros_like(M64)
    seq = []
    for _ in range(n):
        e_lo = rho * D + (flo - M64)
        e_hi = rho * D + (fhi - M64)
        Wq = np.where(np.abs(e_lo) <= np.abs(e_hi), flo, fhi)
        D = rho * D + (Wq - M64)
        seq.append(Wq.astype(np.float32))
    return seq


def _blockdiag(Mt, dtype):
    """lhsT tile: 4-group block-diagonal of Mt (already transposed)."""
    out = np.zeros((128, 128), dtype=dtype)
    for g in range(4):
        out[32 * g:32 * (g + 1), 32 * g:32 * (g + 1)] = Mt
    return out


def _cheby_params(L, mu, n_ramp, n_total):
    d = (L + mu) / 2.0
    cc = (L - mu) / 2.0
    al = [0.0] * (n_total + 1)
    be = [0.0] * (n_total + 1)
    w_prev = 0.0
    for k in range(1, n_total + 1):
        w = 1.0 / d if k == 1 else 1.0 / (d - cc * cc / 4.0 * w_prev)
        be[k] = (cc / 2.0) ** 2 * w_prev * w if k > 1 else 0.0
        al[k] = w
        w_prev = w
    kap = L / mu
    aH = ALPHA_BACKOFF * 4.0 / (np.sqrt(L) + np.sqrt(mu)) ** 2
    bH = ((np.sqrt(kap) - 1.0) / (np.sqrt(kap) + 1.0)) ** 2
    for k in range(n_ramp + 1, n_total + 1):
        al[k] = aH
        be[k] = bH
    return al, be


def host_prep(A: np.ndarray):
    A64 = np.asarray(A, dtype=np.float64)
    AtA = A64.T @ A64
    ev = np.linalg.eigvalsh(AtA)
    L, mu = ev[-1], ev[0]
    I = np.eye(KD)
    al, be = _cheby_params(L, mu, N_RAMP, TOTAL_UPD)

    # ramp tiles: w_1..N_RAMP (whole Wa_k, nearest-bf16), bid_2..N_RAMP
    n_ramp_tiles = N_RAMP + (N_RAMP - 1)
    wramp = np.zeros((128, n_ramp_tiles * 128), dtype=ml_dtypes.bfloat16)
    for k in range(1, N_RAMP + 1):
        bq = _bf16_val(be[k])
        Wa = (1.0 + bq) * I - al[k] * AtA
        wramp[:, 128 * (k - 1):128 * k] = _blockdiag(
            _bf16(Wa.T), ml_dtypes.bfloat16)
        if k >= 2:
            bid = np.zeros((32, 32), dtype=np.float64)
            np.fill_diagonal(bid, -bq)
            wramp[:, 128 * (N_RAMP + k - 2):128 * (N_RAMP + k - 1)] = \
                _blockdiag(_bf16(bid), ml_dtypes.bfloat16)

    # constant phase: one bf16-exact beta everywhere, sigma-delta
    # (rho-discounted per-entry error feedback) rounding sequences so the
    # effective weight seen by the fixed point is unbiased even over few
    # iterations (random dither left a ~1e-2 realization lottery).
    aH, bH = al[N_RAMP + 1], be[N_RAMP + 1]
    bHq = _bf16_val(bH)
    WaH = (1.0 + bHq) * I - aH * AtA
    n_w32 = TOTAL_UPD - (N_BF16 + 1)
    wconst = np.zeros((128, (N_CONST + 1) * 128), dtype=ml_dtypes.bfloat16)
    for i, v in enumerate(_sigma_delta(WaH.T, N_CONST, SD_RHO, 7)):
        wconst[:, 128 * i:128 * (i + 1)] = _blockdiag(v, ml_dtypes.bfloat16)
    bidH = np.diag([-bHq] * KD)
    wconst[:, 128 * N_CONST:] = _blockdiag(
        _bf16(bidH), ml_dtypes.bfloat16)

    # fp32r phase: same beta (bf16 value is f32- and fp32r-exact)
    w32 = np.zeros((128, (n_w32 + 1) * 128), dtype=np.float32)
    for i, v in enumerate(_sigma_delta(WaH.T, n_w32, SD_RHO, 11)):
        w32[:, 128 * i:128 * (i + 1)] = _blockdiag(v, np.float32)
    w32[:, 128 * n_w32:] = _blockdiag(
        np.diag([-bHq] * KD).astype(np.float32), np.float32)

    # prologue lhs: -A(AtA)^-1, bf16-rounded with per-column error
    # feedback DOWN THE ROW (contraction) AXIS: since E = Pq.T @ X and
    # X ~ U[0,1) is row-iid, keeping each column's running rounding
    # residual near zero cancels the mean-field bias of E.
    P = A64 @ np.linalg.inv(AtA)
    Pm = -P
    flo, fhi = _grid(Pm, 7)
    Pq = np.zeros_like(Pm)
    D = np.zeros(Pm.shape[1])
    for m in range(Pm.shape[0]):
        e_lo = D + (flo[m] - Pm[m])
        e_hi = D + (fhi[m] - Pm[m])
        take_lo = np.abs(e_lo) <= np.abs(e_hi)
        Pq[m] = np.where(take_lo, flo[m], fhi[m])
        D = np.where(take_lo, e_lo, e_hi)
    ppad = np.zeros((128, 16 * 128), dtype=ml_dtypes.bfloat16)
    for cch in range(4):
        pv = Pq[128 * cch:128 * (cch + 1), :]
        for g in range(4):
            ppad[:, 128 * (4 * cch + g) + 32 * g:
                 128 * (4 * cch + g) + 32 * (g + 1)] = _bf16(pv)
    return wramp, ppad, wconst, w32


_PROGRAM_CACHE = {}


def _get_program(ns):
    if ns not in _PROGRAM_CACHE:
        _PROGRAM_CACHE[ns] = build_program(ns)
    return _PROGRAM_CACHE[ns]


def kernel(X: np.ndarray, A: np.ndarray) -> np.ndarray:
    global LAST_RESULTS
    X = np.ascontiguousarray(np.asarray(X, dtype=np.float32))
    A = np.ascontiguousarray(np.asarray(A, dtype=np.float32))
    assert X.shape == (M, N_FULL) and A.shape == (M, KD)

    ns = N_FULL // N_CORES
    wramp, ppad, wconst, w32 = host_prep(A)
    nc = _get_program(ns)

    Xb = _bf16(X)
    wr1 = np.ascontiguousarray(wramp[:, 0:128])
    in_maps = []
    for c in range(N_CORES):
        in_maps.append({
            "x": np.ascontiguousarray(Xb[:, c * ns:(c + 1) * ns]),
            "wr1": wr1,
            "wramp": wramp,
            "ppad": ppad,
            "wconst": wconst,
            "w32": w32,
        })

    res = run_bass_kernel_spmd(nc, in_maps, core_ids=list(range(N_CORES)))
    LAST_RESULTS = res
    S = np.concatenate([res.results[c]["s_out"] for c in range(N_CORES)],
                       axis=1)
    return np.ascontiguousarray(S.astype(np.float32))
